# revision 14
# baseline (speedup 1.0000x reference)
"""Trainium2 Bass kernel for nn_Net_LSV: neural local-stochastic-vol Monte Carlo.

Data-parallel over MC paths across 8 NeuronCores (2048 paths/core).
Layout per core: path p = g*128 + i -> partition i, chunk g (g in [0,16)).
Per-step pipeline:
  - state (slog, v) kept path-major [128,16] f32 + bf16 staging [128,32]
  - 8 PE transposes (bf16) -> feature-major x2 [6,1024] (slog/v/ones x 2 halves)
  - L1 (sv 100 + vh/vdvv 120 fused) via time-indexed weight tables (bias baked
    into a third input row -> relus are bias-free, runnable on any engine)
  - L2 sv 100x100, then per-chunk folds: pd ([101,128] x [101,1]) and
    vvh ([121,128] x [121,86] -> 84 hedge cols + vd + vv)
  - softplus / tamed-Euler state update spread across Act/DVE/Pool
Matmul operands bf16 (f32 psum accumulate); elementwise f32.
"""
import numpy as np
from contextlib import ExitStack

import concourse.bass as bass
import concourse.bacc as bacc
import concourse.tile as tile
from concourse import mybir
from concourse.masks import make_identity
from concourse.bass_utils import run_bass_kernel_spmd
import concourse.hw_specs as hw_specs_mod

F32 = mybir.dt.float32
BF16 = mybir.dt.bfloat16
AF = mybir.ActivationFunctionType
OP = mybir.AluOpType

N_CORES = 8
MC = 16384
P = 128
G = 16
MCC = P * G            # paths per core
NS = 21                # strikes
NM = 4                 # maturities
H1 = 100               # s_vol hidden
VH = 20                # vanilla hedge hidden (per maturity)
DV = 20                # v_drift / v_vol hidden
WB = NM * VH + 2 * DV  # fused vh+vdvv L1 width (120)
NF = NM * NS           # 84 hedge outputs
SQH_HALF = float(np.sqrt(0.5))

_ONE_TABLE = "natural_log_exp_and_others"
_MY_FNS = None


def _patched_tables_factory(orig):
    def patched(arch):
        t = orig(arch)
        if _ONE_TABLE not in t:
            return t
        global _MY_FNS
        if _MY_FNS is None:
            _MY_FNS = {
                AF.Relu, AF.Exp, AF.Ln, AF.Abs, AF.Copy, AF.Square,
                AF.Identity,
            }
        out = {}
        for k, v in t.items():
            out[k] = v if k == _ONE_TABLE else (v - _MY_FNS)
        return out
    return patched


def build_program(steps):
    T = len(steps)
    n_ev = sum(1 for s in steps if s["event"] is not None)
    nc = bacc.Bacc()

    # ---------------- DRAM I/O ----------------
    z_d = nc.declare_dram_parameter("z_land", [P, T, G], F32, isOutput=False)
    db_d = nc.declare_dram_parameter("db_land", [P, T, G], F32, isOutput=False)
    w1sv_d = nc.declare_dram_parameter("w1sv_tab", [3, T * H1], BF16, isOutput=False)
    w1vh_d = nc.declare_dram_parameter("w1vh_tab", [3, T * WB], BF16, isOutput=False)
    w2aug_d = nc.declare_dram_parameter("w2aug", [H1 + 1, H1], BF16, isOutput=False)
    w3b_d = nc.declare_dram_parameter("w3b", [H1 + 1, 1], BF16, isOutput=False)
    wvvh_d = nc.declare_dram_parameter("wvvh", [WB + 1, NF + 2], BF16, isOutput=False)
    expb_d = nc.declare_dram_parameter("expb", [1, T], F32, isOutput=False)
    krep_d = nc.declare_dram_parameter("krep", [1, max(n_ev, 1) * NS], F32, isOutput=False)
    init_d = nc.declare_dram_parameter("initvals", [1, 4], F32, isOutput=False)
    out_d = nc.declare_dram_parameter("out", [2 * NM * NS], F32, isOutput=True)

    with tile.TileContext(nc) as tc, ExitStack() as ctx:
        stat = ctx.enter_context(tc.tile_pool(name="stat", bufs=1))
        work = ctx.enter_context(tc.tile_pool(name="work", bufs=2))
        spw = ctx.enter_context(tc.tile_pool(name="spw", bufs=1))
        ps_x2 = ctx.enter_context(tc.tile_pool(name="ps_x2", bufs=1, space="PSUM"))
        ps_mlp = ctx.enter_context(tc.tile_pool(name="ps_mlp", bufs=3, space="PSUM"))
        ps_pd = ctx.enter_context(tc.tile_pool(name="ps_pd", bufs=1, space="PSUM"))
        ps_cvf = ctx.enter_context(tc.tile_pool(name="ps_cvf", bufs=2, space="PSUM"))

        # ---------- static tiles ----------
        ident_f = stat.tile([P, P], F32)
        make_identity(nc, ident_f[:])
        ident = stat.tile([P, P], BF16)
        nc.vector.tensor_copy(ident[:], ident_f[:])

        zt = stat.tile([P, T, G], F32)
        nc.sync.dma_start(out=zt[:], in_=z_d[:])
        dbt = stat.tile([P, T, G], F32)
        nc.sync.dma_start(out=dbt[:], in_=db_d[:])

        def load_bf16(dram, shape, tag):
            b = stat.tile(shape, BF16, tag=tag + "_b")
            nc.sync.dma_start(out=b[:], in_=dram[:])
            return b

        def load_bf16_dup32(dram, shape, tag):
            b = stat.tile([32 + shape[0]] + shape[1:], BF16, tag=tag + "_b")
            nc.sync.dma_start(out=b[0:shape[0]], in_=dram[:])
            nc.sync.dma_start(out=b[32:32 + shape[0]], in_=dram[:])
            return b

        w1sv = load_bf16_dup32(w1sv_d, [3, T * H1], "w1sv")
        w1vh = load_bf16_dup32(w1vh_d, [3, T * WB], "w1vh")
        w2aug = load_bf16(w2aug_d, [H1 + 1, H1], "w2aug")
        w3b = load_bf16(w3b_d, [H1 + 1, 1], "w3b")
        wvvh = load_bf16(wvvh_d, [WB + 1, NF + 2], "wvvh")

        expb = stat.tile([P, T], F32)
        nc.sync.dma_start(out=expb[:], in_=expb_d[:].broadcast_to([P, T]))
        krep = stat.tile([P, max(n_ev, 1) * NS], F32)
        nc.sync.dma_start(out=krep[:], in_=krep_d[:].broadcast_to([P, max(n_ev, 1) * NS]))
        initv = stat.tile([P, 4], F32)
        nc.sync.dma_start(out=initv[:], in_=init_d[:].broadcast_to([P, 4]))

        bias0 = stat.tile([P, 1], F32)
        nc.gpsimd.memset(bias0[:], 0.0)
        ones_col = stat.tile([P, 1], F32)
        nc.gpsimd.memset(ones_col[:], 1.0)

        # ---------- persistent state ----------
        slog = stat.tile([P, G], F32)
        nc.vector.tensor_copy(slog[:], initv[:, 0:1].broadcast_to([P, G]))
        v = stat.tile([P, G], F32)
        nc.vector.tensor_copy(v[:], initv[:, 1:2].broadcast_to([P, G]))
        sd_a = stat.tile([P, G], F32)
        nc.vector.tensor_copy(sd_a[:], initv[:, 2:3].broadcast_to([P, G]))
        sd_b = stat.tile([P, G], F32)
        nc.gpsimd.memset(sd_b[:], 0.0)
        # bf16 staging: cols 0:16 slog chunks, 16:32 v chunks
        svb = stat.tile([P, 2 * G], BF16)

        # feature-major input: rows 0:3 (slogA, vA, 1), rows 32:35 (slogB, vB, 1)
        x2sb = stat.tile([35, MCC // 2], BF16)
        nc.gpsimd.memset(x2sb[:], 1.0)

        h1sv = stat.tile([H1 + 1, MCC], BF16)
        nc.gpsimd.memset(h1sv[:], 1.0)
        h1vh = stat.tile([WB + 1, MCC], BF16)
        nc.gpsimd.memset(h1vh[:], 1.0)
        h2aug = stat.tile([H1 + 1, MCC], BF16)
        nc.gpsimd.memset(h2aug[:], 1.0)

        cvpre_a = stat.tile([P, NM, G, NS], F32)
        cvpre_b = stat.tile([P, NM, G, NS], F32)
        cvpre_t = [cvpre_a, cvpre_b]
        cvfwd = stat.tile([P, NM, G, NS], F32)
        dS_a = stat.tile([P, G], F32)
        dS_b = stat.tile([P, G], F32)
        dS_t = [dS_a, dS_b]
        cv = stat.tile([P, NM, G, NS], F32)
        nc.gpsimd.memset(cv[:], 0.0)
        vdvu = stat.tile([P, 2 * G], F32)    # cols 0:16 vd, 16:32 vv(pre-softplus)
        pd = stat.tile([P, G], F32)
        vvol = stat.tile([P, G], F32)
        outacc = stat.tile([1, 2 * NM * NS], F32)
        nc.gpsimd.memset(outacc[:], 0.0)

        sd_tiles = [sd_a, sd_b]

        # round-robin engine pickers
        eng3 = [nc.scalar, nc.vector, nc.gpsimd]
        eng2l = [nc.scalar, nc.vector]

        def relu_on(eng, out, in_, nparts):
            if eng is nc.scalar:
                nc.scalar.activation(out, in_, AF.Relu,
                                     bias=bias0[0:nparts, :], scale=1.0)
            else:
                eng.tensor_scalar(out, in_, 0.0, None, OP.max)

        def copy_on(eng, out, in_):
            if eng is nc.scalar:
                nc.scalar.copy(out, in_)
            else:
                eng.tensor_copy(out, in_)

        def sp_softplus(dst, src, nparts, tagp, abs_eng, r_eng, add_eng,
                        abs_act=False):
            """dst = softplus(src), stable: max(x,0) + ln(1+exp(-|x|))."""
            shape = list(src.shape)
            pool = spw if tagp == "sp_cv" else work
            a = pool.tile(shape, F32, tag=tagp + "_a")
            if abs_act:
                nc.scalar.activation(a[:], src, AF.Abs,
                                     bias=bias0[0:nparts, :], scale=1.0)
            else:
                abs_eng.scalar_tensor_tensor(a[:], src, -1.0, src, OP.mult, OP.max)
            e = pool.tile(shape, F32, tag=tagp + "_e")
            nc.scalar.activation(e[:], a[:], AF.Exp, bias=bias0[0:nparts, :], scale=-1.0)
            l = pool.tile(shape, F32, tag=tagp + "_l")
            nc.scalar.activation(l[:], e[:], AF.Ln, bias=ones_col[0:nparts, :], scale=1.0)
            r = pool.tile(shape, F32, tag=tagp + "_r")
            r_eng.tensor_scalar(r[:], src, 0.0, None, OP.max)
            add_eng.tensor_tensor(dst, r[:], l[:], OP.add)

        def emit_deferred(tp):
            """Bulk cv work of step tp: softplus, cv update, maturity event."""
            st = steps[tp]
            idx = st["idx"]
            nlive = NM - idx
            cvpre = cvpre_t[tp % 2]
            dS = dS_t[tp % 2]
            sp_softplus(cvfwd[:, idx:NM].rearrange("p k g s -> p (k g s)"),
                        cvpre[:, idx:NM].rearrange("p k g s -> p (k g s)"),
                        P, "sp_cv", nc.vector, nc.vector, nc.gpsimd)
            dS_b = dS[:].unsqueeze(1).unsqueeze(-1).broadcast_to([P, nlive, G, NS])
            cvds = work.tile([P, NM, G, NS], F32, tag="cvds")
            nc.vector.tensor_tensor(cvds[:, idx:NM], cvfwd[:, idx:NM], dS_b, OP.mult)
            nc.gpsimd.tensor_tensor(cv[:, idx:NM], cv[:, idx:NM], cvds[:, idx:NM],
                                    OP.add)
            if st["event"] is not None:
                ev, kslots = st["event"]
                sd_new = sd_tiles[(tp + 1) % 2]
                pay = work.tile([P, G, NS], F32, tag="pay")
                sd_bc = sd_new[:].unsqueeze(-1).broadcast_to([P, G, NS])
                kd_bc = krep[:, ev * NS:(ev + 1) * NS].unsqueeze(1).broadcast_to([P, G, NS])
                nc.gpsimd.tensor_tensor(pay[:], sd_bc, kd_bc, OP.subtract)
                nc.vector.tensor_scalar(pay[:], pay[:], 0.0, None, OP.max)
                price = work.tile([P, G, NS], F32, tag="price")
                nc.gpsimd.tensor_tensor(price[:], pay[:], cv[:, idx, :, :],
                                        OP.subtract)
                price2 = work.tile([P, G, NS], F32, tag="price2")
                nc.gpsimd.tensor_tensor(price2[:], price[:], price[:], OP.mult)
                red = work.tile([P, 2 * NS], F32, tag="red")
                nc.vector.tensor_reduce(red[:, 0:NS], price[:].transpose([0, 2, 1]),
                                        mybir.AxisListType.X, OP.add)
                nc.vector.tensor_reduce(red[:, NS:2 * NS], price2[:].transpose([0, 2, 1]),
                                        mybir.AxisListType.X, OP.add)
                pred = ps_pd.tile([1, 2 * NS], F32, tag="pdps")
                nc.tensor.matmul(pred[:], ones_col[:], red[:])
                for k in kslots:
                    nc.scalar.copy(outacc[0:1, k * NS:(k + 1) * NS], pred[0:1, 0:NS])
                    nc.scalar.copy(outacc[0:1, NM * NS + k * NS:NM * NS + (k + 1) * NS],
                                   pred[0:1, NS:2 * NS])

        for t, st in enumerate(steps):
            h, sqh = st["h"], st["sqh"]
            rate = st["rate"]
            idx = st["idx"]
            nlive = NM - idx
            cvpre = cvpre_t[t % 2]
            dS = dS_t[t % 2]
            sd_old = sd_tiles[t % 2]
            sd_new = sd_tiles[(t + 1) % 2]
            z_t = zt[:, t, :]        # sqh * z  (host-scaled)
            db_t = dbt[:, t, :]      # rho_s*z + c_s*zz (host-scaled)

            # ---- bf16 staging of state (2 contiguous cast copies) ----
            nc.vector.tensor_copy(svb[:, 0:G], slog[:])
            nc.scalar.copy(svb[:, G:2 * G], v[:])

            # ---- 16 PE transposes -> x2psA (chunks 0-7) / x2psB (8-15) ----
            x2psA = ps_x2.tile([2, MCC // 2], BF16, tag="x2psA")
            x2psB = ps_x2.tile([2, MCC // 2], BF16, tag="x2psB")
            for q in range(8):
                nc.tensor.transpose(x2psA[:, q * P:(q + 1) * P],
                                    svb[:, q:2 * G:G], ident[:])
            nc.scalar.copy(x2sb[0:2, 0:512], x2psA[:, 0:512])
            nc.vector.tensor_copy(x2sb[0:2, 512:1024], x2psA[:, 512:1024])
            for q in range(8):
                nc.tensor.transpose(x2psB[:, q * P:(q + 1) * P],
                                    svb[:, 8 + q:2 * G:G], ident[:])
            nc.scalar.copy(x2sb[32:34, 0:512], x2psB[:, 0:512])
            nc.vector.tensor_copy(x2sb[32:34, 512:1024], x2psB[:, 512:1024])

            # ---- L1 sv + relu, L2 + relu, pd folds, softplus(pd) ----
            l2ps = []
            for q in range(4):
                m = ps_mlp.tile([P, 512], F32, tag="mlp")
                if q < 2:
                    rhs = x2sb[0:3, q * 512:(q + 1) * 512]
                    lhsT = w1sv[0:3, t * H1:(t + 1) * H1]
                else:
                    rhs = x2sb[32:35, (q - 2) * 512:(q - 1) * 512]
                    lhsT = w1sv[32:35, t * H1:(t + 1) * H1]
                nc.tensor.matmul(m[0:H1, :], lhsT, rhs)
                relu_on(eng2l[q % 2], h1sv[0:H1, q * 512:(q + 1) * 512],
                        m[0:H1, :], H1)
            for q in range(4):
                m = ps_mlp.tile([P, 512], F32, tag="mlp")
                nc.tensor.matmul(m[0:H1, :], w2aug[:],
                                 h1sv[:, q * 512:(q + 1) * 512])
                relu_on(eng2l[(q + 1) % 2], h2aug[0:H1, q * 512:(q + 1) * 512],
                        m[0:H1, :], H1)
            pdps = ps_pd.tile([P, G], F32, tag="pdps")
            for g in range(G):
                nc.tensor.matmul(pdps[:, g:g + 1],
                                 h2aug[:, g * P:(g + 1) * P], w3b[:])
            sp_softplus(pd[:], pdps[:], P, "sp_pd", nc.vector, nc.vector,
                        nc.vector, abs_act=True)

            # ---- Slog update chain ----
            sq = work.tile([P, G], F32, tag="sq")
            nc.scalar.activation(sq[:], pd[:], AF.Square, bias=bias0[:],
                                 scale=SQH_HALF)
            drift = work.tile([P, G], F32, tag="drift")
            nc.vector.tensor_scalar(drift[:], sq[:], -1.0, float(rate),
                                    OP.mult, OP.add)
            dc = work.tile([P, G], F32, tag="dc")
            nc.scalar.activation(dc[:], drift[:], AF.Abs, bias=bias0[:],
                                 scale=float(sqh))
            nc.vector.tensor_scalar(dc[:], dc[:], 1.0, None, OP.add)
            rcp1 = work.tile([P, G], F32, tag="rcp1")
            nc.vector.reciprocal(rcp1[:], dc[:])
            term1 = work.tile([P, G], F32, tag="term1")
            nc.vector.scalar_tensor_tensor(term1[:], drift[:], float(h), rcp1[:],
                                           OP.mult, OP.mult)
            fc = work.tile([P, G], F32, tag="fc")
            nc.vector.tensor_scalar(fc[:], pd[:], float(sqh), 1.0, OP.mult, OP.add)
            rcp2 = work.tile([P, G], F32, tag="rcp2")
            nc.vector.reciprocal(rcp2[:], fc[:])
            diffz = work.tile([P, G], F32, tag="diffz")
            nc.vector.tensor_tensor(diffz[:], z_t, pd[:], OP.mult)
            term2 = work.tile([P, G], F32, tag="term2")
            nc.vector.tensor_tensor(term2[:], diffz[:], rcp2[:], OP.mult)
            t12 = work.tile([P, G], F32, tag="t12")
            nc.vector.tensor_tensor(t12[:], term1[:], term2[:], OP.add)
            nc.vector.tensor_tensor(slog[:], slog[:], t12[:], OP.add)
            nc.scalar.activation(sd_new[:], slog[:], AF.Exp,
                                 bias=expb[:, t:t + 1], scale=1.0)
            nc.vector.tensor_tensor(dS[:], sd_new[:], sd_old[:], OP.subtract)

            # ---- vh/vdvv L1 + relu, vvh folds (2 chunks per psum tile) ----
            for q in range(4):
                m = ps_mlp.tile([P, 512], F32, tag="mlp")
                if q < 2:
                    rhs = x2sb[0:3, q * 512:(q + 1) * 512]
                    lhsT = w1vh[0:3, t * WB:(t + 1) * WB]
                else:
                    rhs = x2sb[32:35, (q - 2) * 512:(q - 1) * 512]
                    lhsT = w1vh[32:35, t * WB:(t + 1) * WB]
                nc.tensor.matmul(m[0:WB, :], lhsT, rhs)
                relu_on(eng2l[q % 2], h1vh[0:WB, q * 512:(q + 1) * 512],
                        m[0:WB, :], WB)
            NC2 = NF + 2
            for gp in range(G // 2):
                cvk = ps_cvf.tile([P, 2 * NC2], F32, tag="cvk")
                nc.tensor.matmul(cvk[:, 0:NC2],
                                 h1vh[:, (2 * gp) * P:(2 * gp + 1) * P], wvvh[:])
                nc.tensor.matmul(cvk[:, NC2:2 * NC2],
                                 h1vh[:, (2 * gp + 1) * P:(2 * gp + 2) * P], wvvh[:])
                src_cv = cvk[:].rearrange("p (j f) -> p j f", j=2)[:, :, idx * NS:NF] \
                    .rearrange("p j (k s) -> p k j s", k=nlive)
                copy_on(eng2l[gp % 2], cvpre[:, idx:NM, 2 * gp:2 * gp + 2, :], src_cv)
                src_vd = cvk[:].rearrange("p (j f) -> p j f", j=2)[:, :, NF:NF + 2] \
                    .rearrange("p j d -> p d j")
                dst_vd = vdvu[:].rearrange("p (d g) -> p d g", d=2)[:, :, 2 * gp:2 * gp + 2]
                copy_on(eng2l[(gp + 1) % 2], dst_vd, src_vd)

            # ---- softplus(vv) and V update ----
            sp_softplus(vvol[:], vdvu[:, G:2 * G], P, "sp_vv", nc.vector,
                        nc.vector, nc.gpsimd)
            vtmp = work.tile([P, G], F32, tag="vtmp")
            nc.vector.scalar_tensor_tensor(vtmp[:], vdvu[:, 0:G], float(h), v[:],
                                           OP.mult, OP.add)
            vvdB = work.tile([P, G], F32, tag="vvdB")
            nc.vector.tensor_tensor(vvdB[:], vvol[:], db_t, OP.mult)
            nc.vector.tensor_tensor(v[:], vtmp[:], vvdB[:], OP.add)

            # ---- deferred bulk of previous step ----
            if t > 0:
                emit_deferred(t - 1)
        emit_deferred(T - 1)

        nc.sync.dma_start(out=out_d[:].unsqueeze(0), in_=outacc[:])

    orig = bacc.get_activation_tables
    try:
        bacc.get_activation_tables = _patched_tables_factory(
            hw_specs_mod.get_activation_tables)
        nc.compile()
    finally:
        bacc.get_activation_tables = orig
    return nc


def _prep(inputs):
    """Host-side preprocessing -> (steps, arrays-for-in_maps, shards, meta)."""
    f = lambda k: np.asarray(inputs[k], dtype=np.float32)
    S0 = float(f("S0")); rate = float(f("rate"))
    z = f("z"); zz = f("zz")
    timegrid = f("timegrid"); strikes = f("strikes")
    v0 = float(f("v0")[0]); rho = float(f("rho")[0])
    mats = np.asarray(inputs["maturities"]).astype(np.int64)

    rho_t = float(np.tanh(np.float32(rho)))
    c_t = float(np.sqrt(np.float32(1.0) - np.float32(rho_t) ** 2))
    V0 = float(1.0 / (1.0 + np.exp(-np.float32(v0))) * 0.5)
    slog0 = float(np.log(np.float32(S0)))

    days = np.round(timegrid * 365.0).astype(np.int64)
    le = days[1:, None] <= mats[None, :]
    idx_net = np.argmax(le, axis=1)
    is_mat = np.any(days[1:, None] == mats[None, :], axis=1)

    if not is_mat.any():
        return None

    T = int(np.max(np.nonzero(is_mat)[0])) + 1
    steps = []
    krep_list = []
    ev = 0
    for t in range(T):
        t0 = float(timegrid[t]); t1 = float(timegrid[t + 1])
        h = float(np.float32(t1) - np.float32(t0))
        sqh = float(np.sqrt(np.float32(h)))
        event = None
        if is_mat[t]:
            k = int(idx_net[t])
            event = (ev, [k])
            krep_list.append(np.exp(-rate * t1).astype(np.float32) * strikes)
            ev += 1
        steps.append(dict(
            t0=t0, h=h, sqh=sqh, rho_s=rho_t * sqh, c_s=c_t * sqh, rate=rate,
            idx=int(idx_net[t]), event=event,
        ))

    sv_W1 = f("sv_W1"); sv_b1 = f("sv_b1"); sv_W2 = f("sv_W2"); sv_b2 = f("sv_b2")
    sv_W3 = f("sv_W3"); sv_b3 = f("sv_b3")
    vh_W1 = f("vh_W1"); vh_b1 = f("vh_b1"); vh_W2 = f("vh_W2"); vh_b2 = f("vh_b2")
    vd_W1 = f("vd_W1"); vd_b1 = f("vd_b1"); vd_W2 = f("vd_W2"); vd_b2 = f("vd_b2")
    vv_W1 = f("vv_W1"); vv_b1 = f("vv_b1"); vv_W2 = f("vv_W2"); vv_b2 = f("vv_b2")

    t0s = timegrid[:T].astype(np.float32)

    arrs = {}
    # w1sv_tab [3, T*H1]: rows (w_slog, w_v, b1 + w_t0*t0)
    tab = np.zeros((3, T, H1), np.float32)
    tab[0] = sv_W1[1]
    tab[1] = sv_W1[2]
    tab[2] = sv_b1[None, :] + np.outer(t0s, sv_W1[0])
    arrs["w1sv_tab"] = tab.reshape(3, T * H1)

    # w1vh_tab [3, T*WB]: cols 0:80 vh (slog-in), 80:120 vdvv (v-in)
    tab = np.zeros((3, T, WB), np.float32)
    tab[0, :, 0:NM * VH] = vh_W1[:, 1, :].reshape(NM * VH)[None, :]
    tab[1, :, NM * VH:NM * VH + DV] = vd_W1[0][None, :]
    tab[1, :, NM * VH + DV:WB] = vv_W1[0][None, :]
    tab[2, :, 0:NM * VH] = (vh_b1.reshape(NM * VH)[None, :]
                            + np.outer(t0s, vh_W1[:, 0, :].reshape(NM * VH)))
    tab[2, :, NM * VH:NM * VH + DV] = vd_b1[None, :]
    tab[2, :, NM * VH + DV:WB] = vv_b1[None, :]
    arrs["w1vh_tab"] = tab.reshape(3, T * WB)

    arrs["w2aug"] = np.concatenate([sv_W2, sv_b2[None, :]], 0)
    arrs["w3b"] = np.concatenate([sv_W3, sv_b3[None, :]], 0)

    wvvh = np.zeros((WB + 1, NF + 2), np.float32)
    for k in range(NM):
        wvvh[k * VH:(k + 1) * VH, k * NS:(k + 1) * NS] = vh_W2[k]
        wvvh[WB, k * NS:(k + 1) * NS] = vh_b2[k]
    wvvh[NM * VH:NM * VH + DV, NF] = vd_W2[:, 0]
    wvvh[WB, NF] = vd_b2[0]
    wvvh[NM * VH + DV:WB, NF + 1] = vv_W2[:, 0]
    wvvh[WB, NF + 1] = vv_b2[0]
    arrs["wvvh"] = wvvh

    t1s = timegrid[1:T + 1]
    arrs["expb"] = np.ascontiguousarray((-rate * t1s)[None, :].astype(np.float32))
    if krep_list:
        arrs["krep"] = np.concatenate(krep_list)[None, :].astype(np.float32)
    else:
        arrs["krep"] = np.zeros((1, NS), np.float32)
    sd0 = float(np.exp(np.float32(slog0) - np.float32(rate) * timegrid[0]))
    arrs["initvals"] = np.array([[slog0, V0, sd0, 0.0]], np.float32)
    import ml_dtypes
    bf16_keys = {"w1sv_tab", "w1vh_tab", "w2aug", "w3b", "wvvh"}
    for k in arrs:
        dt = ml_dtypes.bfloat16 if k in bf16_keys else np.float32
        arrs[k] = np.ascontiguousarray(arrs[k].astype(dt))

    # host-prescaled noise: z_land = sqh*z ; db_land = rho_s*z + c_s*zz
    sqh_v = np.array([s["sqh"] for s in steps], np.float32)[None, :]
    rho_v = np.array([s["rho_s"] for s in steps], np.float32)[None, :]
    cs_v = np.array([s["c_s"] for s in steps], np.float32)[None, :]
    zshards, dbshards = [], []
    for c in range(N_CORES):
        zc = z[c * MCC:(c + 1) * MCC, :T]
        zzc = zz[c * MCC:(c + 1) * MCC, :T]
        zs = zc * sqh_v
        dbs = zc * rho_v + zzc * cs_v
        for src, lst in ((zs, zshards), (dbs, dbshards)):
            s = src.reshape(G, P, T).transpose(1, 2, 0)
            lst.append(np.ascontiguousarray(s, dtype=np.float32))

    written = sorted({k for s in steps if s["event"] for k in s["event"][1]})
    return steps, arrs, zshards, dbshards, written, T


_CACHE = {}


def kernel(**inputs) -> np.ndarray:
    prep = _prep(inputs)
    if prep is None:
        return np.zeros((2, NM, NS), np.float32)
    steps, arrs, zshards, dbshards, written, T = prep

    key = (T,) + tuple(
        (s["t0"], s["h"], s["rho_s"], s["c_s"], s["rate"], s["idx"],
         None if s["event"] is None else (s["event"][0], tuple(s["event"][1])))
        for s in steps)
    nc = _CACHE.get(key)
    if nc is None:
        nc = build_program(steps)
        _CACHE[key] = nc

    in_maps = []
    for c in range(N_CORES):
        m = dict(arrs)
        m["z_land"] = zshards[c]
        m["db_land"] = dbshards[c]
        in_maps.append(m)

    res = run_bass_kernel_spmd(nc, in_maps, list(range(N_CORES)))
    sums = np.zeros(2 * NM * NS, np.float64)
    for c in range(N_CORES):
        sums += res.results[c]["out"].astype(np.float64)
    s1 = sums[:NM * NS].reshape(NM, NS)
    s2 = sums[NM * NS:].reshape(NM, NS)
    pv = np.zeros((NM, NS), np.float64)
    pvar = np.zeros((NM, NS), np.float64)
    for k in written:
        pv[k] = s1[k] / MC
        pvar[k] = (s2[k] - MC * pv[k] ** 2) / (MC - 1)
    return np.stack([pv, pvar]).astype(np.float32)


# revision 15
# speedup vs baseline: 1.0092x; 1.0092x over previous
"""Trainium2 Bass kernel for nn_Net_LSV: neural local-stochastic-vol Monte Carlo.

Data-parallel over MC paths across 8 NeuronCores (2048 paths/core).
Layout per core: path p = g*128 + i -> partition i, chunk g (g in [0,16)).
Per-step pipeline:
  - state (slog, v) kept path-major [128,16] f32 + bf16 staging [128,32]
  - 8 PE transposes (bf16) -> feature-major x2 [6,1024] (slog/v/ones x 2 halves)
  - L1 (sv 100 + vh/vdvv 120 fused) via time-indexed weight tables (bias baked
    into a third input row -> relus are bias-free, runnable on any engine)
  - L2 sv 100x100, then per-chunk folds: pd ([101,128] x [101,1]) and
    vvh ([121,128] x [121,86] -> 84 hedge cols + vd + vv)
  - softplus / tamed-Euler state update spread across Act/DVE/Pool
Matmul operands bf16 (f32 psum accumulate); elementwise f32.
"""
import numpy as np
from contextlib import ExitStack

import concourse.bass as bass
import concourse.bacc as bacc
import concourse.tile as tile
from concourse import mybir
from concourse.masks import make_identity
from concourse.bass_utils import run_bass_kernel_spmd
import concourse.hw_specs as hw_specs_mod

F32 = mybir.dt.float32
BF16 = mybir.dt.bfloat16
AF = mybir.ActivationFunctionType
OP = mybir.AluOpType

N_CORES = 8
MC = 16384
P = 128
G = 16
MCC = P * G            # paths per core
NS = 21                # strikes
NM = 4                 # maturities
H1 = 100               # s_vol hidden
VH = 20                # vanilla hedge hidden (per maturity)
DV = 20                # v_drift / v_vol hidden
WB = NM * VH + 2 * DV  # fused vh+vdvv L1 width (120)
NF = NM * NS           # 84 hedge outputs
SQH_HALF = float(np.sqrt(0.5))

_ONE_TABLE = "natural_log_exp_and_others"
_MY_FNS = None


def _patched_tables_factory(orig):
    def patched(arch):
        t = orig(arch)
        if _ONE_TABLE not in t:
            return t
        global _MY_FNS
        if _MY_FNS is None:
            _MY_FNS = {
                AF.Relu, AF.Exp, AF.Ln, AF.Abs, AF.Copy, AF.Square,
                AF.Identity,
            }
        out = {}
        for k, v in t.items():
            out[k] = v if k == _ONE_TABLE else (v - _MY_FNS)
        return out
    return patched


def build_program(steps):
    T = len(steps)
    n_ev = sum(1 for s in steps if s["event"] is not None)
    nc = bacc.Bacc()

    # ---------------- DRAM I/O ----------------
    z_d = nc.declare_dram_parameter("z_land", [P, T, G], F32, isOutput=False)
    db_d = nc.declare_dram_parameter("db_land", [P, T, G], F32, isOutput=False)
    w1sv_d = nc.declare_dram_parameter("w1sv_tab", [3, T * H1], BF16, isOutput=False)
    w1vh_d = nc.declare_dram_parameter("w1vh_tab", [3, T * WB], BF16, isOutput=False)
    w2aug_d = nc.declare_dram_parameter("w2aug", [H1 + 1, H1], BF16, isOutput=False)
    w3b_d = nc.declare_dram_parameter("w3b", [H1 + 1, 1], BF16, isOutput=False)
    wvvh_d = nc.declare_dram_parameter("wvvh", [WB + 1, NF + 2], BF16, isOutput=False)
    expb_d = nc.declare_dram_parameter("expb", [1, T], F32, isOutput=False)
    krep_d = nc.declare_dram_parameter("krep", [1, max(n_ev, 1) * NS], F32, isOutput=False)
    init_d = nc.declare_dram_parameter("initvals", [1, 4], F32, isOutput=False)
    out_d = nc.declare_dram_parameter("out", [2 * NM * NS], F32, isOutput=True)

    with tile.TileContext(nc) as tc, ExitStack() as ctx:
        stat = ctx.enter_context(tc.tile_pool(name="stat", bufs=1))
        work = ctx.enter_context(tc.tile_pool(name="work", bufs=2))
        spw = ctx.enter_context(tc.tile_pool(name="spw", bufs=1))
        ps_x2 = ctx.enter_context(tc.tile_pool(name="ps_x2", bufs=1, space="PSUM"))
        ps_mlp = ctx.enter_context(tc.tile_pool(name="ps_mlp", bufs=2, space="PSUM"))
        ps_pd = ctx.enter_context(tc.tile_pool(name="ps_pd", bufs=1, space="PSUM"))
        ps_cvf = ctx.enter_context(tc.tile_pool(name="ps_cvf", bufs=1, space="PSUM"))

        # ---------- static tiles ----------
        ident_f = stat.tile([P, P], F32)
        make_identity(nc, ident_f[:])
        ident = stat.tile([P, P], BF16)
        nc.vector.tensor_copy(ident[:], ident_f[:])

        zt = stat.tile([P, T, G], F32)
        nc.sync.dma_start(out=zt[:], in_=z_d[:])
        dbt = stat.tile([P, T, G], F32)
        nc.sync.dma_start(out=dbt[:], in_=db_d[:])

        def load_bf16(dram, shape, tag):
            b = stat.tile(shape, BF16, tag=tag + "_b")
            nc.sync.dma_start(out=b[:], in_=dram[:])
            return b

        def load_bf16_dup32(dram, shape, tag):
            b = stat.tile([32 + shape[0]] + shape[1:], BF16, tag=tag + "_b")
            nc.sync.dma_start(out=b[0:shape[0]], in_=dram[:])
            nc.sync.dma_start(out=b[32:32 + shape[0]], in_=dram[:])
            return b

        w1sv = load_bf16_dup32(w1sv_d, [3, T * H1], "w1sv")
        w1vh = load_bf16_dup32(w1vh_d, [3, T * WB], "w1vh")
        w2aug = load_bf16(w2aug_d, [H1 + 1, H1], "w2aug")
        w3b = load_bf16(w3b_d, [H1 + 1, 1], "w3b")
        wvvh = load_bf16(wvvh_d, [WB + 1, NF + 2], "wvvh")

        expb = stat.tile([P, T], F32)
        nc.sync.dma_start(out=expb[:], in_=expb_d[:].broadcast_to([P, T]))
        krep = stat.tile([P, max(n_ev, 1) * NS], F32)
        nc.sync.dma_start(out=krep[:], in_=krep_d[:].broadcast_to([P, max(n_ev, 1) * NS]))
        initv = stat.tile([P, 4], F32)
        nc.sync.dma_start(out=initv[:], in_=init_d[:].broadcast_to([P, 4]))

        bias0 = stat.tile([P, 1], F32)
        nc.gpsimd.memset(bias0[:], 0.0)
        ones_col = stat.tile([P, 1], F32)
        nc.gpsimd.memset(ones_col[:], 1.0)

        # ---------- persistent state ----------
        slog = stat.tile([P, G], F32)
        nc.vector.tensor_copy(slog[:], initv[:, 0:1].broadcast_to([P, G]))
        v = stat.tile([P, G], F32)
        nc.vector.tensor_copy(v[:], initv[:, 1:2].broadcast_to([P, G]))
        sd_a = stat.tile([P, G], F32)
        nc.vector.tensor_copy(sd_a[:], initv[:, 2:3].broadcast_to([P, G]))
        sd_b = stat.tile([P, G], F32)
        nc.gpsimd.memset(sd_b[:], 0.0)
        # bf16 staging: cols 0:16 slog chunks, 16:32 v chunks
        svb = stat.tile([P, 2 * G], BF16)

        # feature-major input: rows 0:3 (slogA, vA, 1), rows 32:35 (slogB, vB, 1)
        x2sb = stat.tile([35, MCC // 2], BF16)
        nc.gpsimd.memset(x2sb[:], 1.0)

        h1sv = stat.tile([H1 + 1, MCC], BF16)
        nc.gpsimd.memset(h1sv[:], 1.0)
        h1vh = stat.tile([WB + 1, MCC], BF16)
        nc.gpsimd.memset(h1vh[:], 1.0)
        h2aug = stat.tile([H1 + 1, MCC], BF16)
        nc.gpsimd.memset(h2aug[:], 1.0)

        cvpre = stat.tile([P, NM, G, NS], BF16)
        cvfwd = stat.tile([P, NM, G, NS], BF16)
        dS = stat.tile([P, G], F32)
        cv = stat.tile([P, NM, G, NS], F32)
        nc.gpsimd.memset(cv[:], 0.0)
        vdvu = stat.tile([P, 2 * G], F32)    # cols 0:16 vd, 16:32 vv(pre-softplus)
        pd = stat.tile([P, G], F32)
        vvol = stat.tile([P, G], F32)
        outacc = stat.tile([1, 2 * NM * NS], F32)
        nc.gpsimd.memset(outacc[:], 0.0)

        sd_tiles = [sd_a, sd_b]

        # round-robin engine pickers
        eng3 = [nc.scalar, nc.vector, nc.gpsimd]
        eng2l = [nc.scalar, nc.vector]

        def relu_on(eng, out, in_, nparts):
            if eng is nc.scalar:
                nc.scalar.activation(out, in_, AF.Relu,
                                     bias=bias0[0:nparts, :], scale=1.0)
            else:
                eng.tensor_scalar(out, in_, 0.0, None, OP.max)

        def copy_on(eng, out, in_):
            if eng is nc.scalar:
                nc.scalar.copy(out, in_)
            else:
                eng.tensor_copy(out, in_)

        def sp_softplus(dst, src, nparts, tagp, abs_eng, r_eng, add_eng,
                        abs_act=False):
            """dst = softplus(src), stable: max(x,0) + ln(1+exp(-|x|))."""
            shape = list(src.shape)
            pool = spw if tagp == "sp_cv" else work
            a = pool.tile(shape, F32, tag=tagp + "_a")
            if abs_act:
                nc.scalar.activation(a[:], src, AF.Abs,
                                     bias=bias0[0:nparts, :], scale=1.0)
            else:
                abs_eng.scalar_tensor_tensor(a[:], src, -1.0, src, OP.mult, OP.max)
            e = pool.tile(shape, F32, tag=tagp + "_e")
            nc.scalar.activation(e[:], a[:], AF.Exp, bias=bias0[0:nparts, :], scale=-1.0)
            l = pool.tile(shape, F32, tag=tagp + "_l")
            nc.scalar.activation(l[:], e[:], AF.Ln, bias=ones_col[0:nparts, :], scale=1.0)
            r = pool.tile(shape, F32, tag=tagp + "_r")
            r_eng.tensor_scalar(r[:], src, 0.0, None, OP.max)
            add_eng.tensor_tensor(dst, r[:], l[:], OP.add)

        NC2 = NF + 2

        for t, st in enumerate(steps):
            h, sqh = st["h"], st["sqh"]
            rate = st["rate"]
            idx = st["idx"]
            nlive = NM - idx
            sd_old = sd_tiles[t % 2]
            sd_new = sd_tiles[(t + 1) % 2]
            z_t = zt[:, t, :]        # sqh * z  (host-scaled)
            db_t = dbt[:, t, :]      # rho_s*z + c_s*zz (host-scaled)

            # ---- bf16 staging of state ----
            nc.vector.tensor_copy(svb[:, 0:G], slog[:])
            nc.scalar.copy(svb[:, G:2 * G], v[:])

            # ---- 16 PE transposes -> x2ps [2, 2048]; 2 copies -> x2sb ----
            x2ps = ps_x2.tile([2, MCC], BF16, tag="x2ps")
            for q in range(8):
                nc.tensor.transpose(x2ps[:, q * P:(q + 1) * P],
                                    svb[:, q:2 * G:G], ident[:])
            nc.scalar.copy(x2sb[0:2, :], x2ps[:, 0:1024])
            for q in range(8):
                nc.tensor.transpose(x2ps[:, 1024 + q * P:1024 + (q + 1) * P],
                                    svb[:, 8 + q:2 * G:G], ident[:])
            nc.vector.tensor_copy(x2sb[32:34, :], x2ps[:, 1024:2048])

            # ---- L1 sv (2 half-MMs per relu), L2, pd folds, softplus(pd) ----
            for hf in range(2):
                m = ps_mlp.tile([P, 1024], F32, tag="mlp")
                base, wrow = (0, 0) if hf == 0 else (32, 32)
                nc.tensor.matmul(m[0:H1, 0:512],
                                 w1sv[wrow:wrow + 3, t * H1:(t + 1) * H1],
                                 x2sb[base:base + 3, 0:512])
                nc.tensor.matmul(m[0:H1, 512:1024],
                                 w1sv[wrow:wrow + 3, t * H1:(t + 1) * H1],
                                 x2sb[base:base + 3, 512:1024])
                relu_on(eng2l[hf % 2], h1sv[0:H1, hf * 1024:(hf + 1) * 1024],
                        m[0:H1, :], H1)
            for hf in range(2):
                m = ps_mlp.tile([P, 1024], F32, tag="mlp")
                nc.tensor.matmul(m[0:H1, 0:512], w2aug[:],
                                 h1sv[:, hf * 1024:hf * 1024 + 512])
                nc.tensor.matmul(m[0:H1, 512:1024], w2aug[:],
                                 h1sv[:, hf * 1024 + 512:(hf + 1) * 1024])
                relu_on(eng2l[(hf + 1) % 2], h2aug[0:H1, hf * 1024:(hf + 1) * 1024],
                        m[0:H1, :], H1)
            pdps = ps_pd.tile([P, G], F32, tag="pdps")
            for g in range(G):
                nc.tensor.matmul(pdps[:, g:g + 1],
                                 h2aug[:, g * P:(g + 1) * P], w3b[:])
            sp_softplus(pd[:], pdps[:], P, "sp_pd", nc.vector, nc.vector,
                        nc.vector, abs_act=True)

            # ---- Slog update chain ----
            sq = work.tile([P, G], F32, tag="sq")
            nc.scalar.activation(sq[:], pd[:], AF.Square, bias=bias0[:],
                                 scale=SQH_HALF)
            drift = work.tile([P, G], F32, tag="drift")
            nc.vector.tensor_scalar(drift[:], sq[:], -1.0, float(rate),
                                    OP.mult, OP.add)
            dc = work.tile([P, G], F32, tag="dc")
            nc.scalar.activation(dc[:], drift[:], AF.Abs, bias=bias0[:],
                                 scale=float(sqh))
            nc.vector.tensor_scalar(dc[:], dc[:], 1.0, None, OP.add)
            rcp1 = work.tile([P, G], F32, tag="rcp1")
            nc.vector.reciprocal(rcp1[:], dc[:])
            term1 = work.tile([P, G], F32, tag="term1")
            nc.vector.scalar_tensor_tensor(term1[:], drift[:], float(h), rcp1[:],
                                           OP.mult, OP.mult)
            fc = work.tile([P, G], F32, tag="fc")
            nc.vector.tensor_scalar(fc[:], pd[:], float(sqh), 1.0, OP.mult, OP.add)
            rcp2 = work.tile([P, G], F32, tag="rcp2")
            nc.vector.reciprocal(rcp2[:], fc[:])
            diffz = work.tile([P, G], F32, tag="diffz")
            nc.vector.tensor_tensor(diffz[:], z_t, pd[:], OP.mult)
            term2 = work.tile([P, G], F32, tag="term2")
            nc.vector.tensor_tensor(term2[:], diffz[:], rcp2[:], OP.mult)
            t12 = work.tile([P, G], F32, tag="t12")
            nc.vector.tensor_tensor(t12[:], term1[:], term2[:], OP.add)
            nc.vector.tensor_tensor(slog[:], slog[:], t12[:], OP.add)
            nc.scalar.activation(sd_new[:], slog[:], AF.Exp,
                                 bias=expb[:, t:t + 1], scale=1.0)
            nc.vector.tensor_tensor(dS[:], sd_new[:], sd_old[:], OP.subtract)

            # ---- vh/vdvv L1, vvh folds (2 chunks/psum tile, live cols) ----
            for hf in range(2):
                m = ps_mlp.tile([P, 1024], F32, tag="mlp")
                base, wrow = (0, 0) if hf == 0 else (32, 32)
                nc.tensor.matmul(m[0:WB, 0:512],
                                 w1vh[wrow:wrow + 3, t * WB:(t + 1) * WB],
                                 x2sb[base:base + 3, 0:512])
                nc.tensor.matmul(m[0:WB, 512:1024],
                                 w1vh[wrow:wrow + 3, t * WB:(t + 1) * WB],
                                 x2sb[base:base + 3, 512:1024])
                relu_on(eng2l[hf % 2], h1vh[0:WB, hf * 1024:(hf + 1) * 1024],
                        m[0:WB, :], WB)
            for gp in range(G // 2):
                cvk = ps_cvf.tile([P, 2 * NC2], F32, tag="cvk")
                nc.tensor.matmul(cvk[:, idx * NS:NC2],
                                 h1vh[:, (2 * gp) * P:(2 * gp + 1) * P],
                                 wvvh[:, idx * NS:NC2])
                nc.tensor.matmul(cvk[:, NC2 + idx * NS:2 * NC2],
                                 h1vh[:, (2 * gp + 1) * P:(2 * gp + 2) * P],
                                 wvvh[:, idx * NS:NC2])
                src_cv = cvk[:].rearrange("p (j f) -> p j f", j=2)[:, :, idx * NS:NF] \
                    .rearrange("p j (k s) -> p k j s", k=nlive)
                copy_on(eng2l[gp % 2], cvpre[:, idx:NM, 2 * gp:2 * gp + 2, :], src_cv)
                src_vd = cvk[:].rearrange("p (j f) -> p j f", j=2)[:, :, NF:NF + 2] \
                    .rearrange("p j d -> p d j")
                dst_vd = vdvu[:].rearrange("p (d g) -> p d g", d=2)[:, :, 2 * gp:2 * gp + 2]
                copy_on(eng2l[(gp + 1) % 2], dst_vd, src_vd)

            # ---- softplus(vv) and V update ----
            sp_softplus(vvol[:], vdvu[:, G:2 * G], P, "sp_vv", nc.vector,
                        nc.vector, nc.gpsimd)
            vtmp = work.tile([P, G], F32, tag="vtmp")
            nc.vector.scalar_tensor_tensor(vtmp[:], vdvu[:, 0:G], float(h), v[:],
                                           OP.mult, OP.add)
            vvdB = work.tile([P, G], F32, tag="vvdB")
            nc.vector.tensor_tensor(vvdB[:], vvol[:], db_t, OP.mult)
            nc.vector.tensor_tensor(v[:], vtmp[:], vvdB[:], OP.add)

            # ---- softplus(cv) in bf16, cv update ----
            sp_softplus(cvfwd[:, idx:NM].rearrange("p k g s -> p (k g s)"),
                        cvpre[:, idx:NM].rearrange("p k g s -> p (k g s)"),
                        P, "sp_cv", nc.vector, nc.vector, nc.gpsimd)
            dS_b = dS[:].unsqueeze(1).unsqueeze(-1).broadcast_to([P, nlive, G, NS])
            cvds = work.tile([P, NM, G, NS], F32, tag="cvds")
            nc.vector.tensor_tensor(cvds[:, idx:NM], cvfwd[:, idx:NM], dS_b, OP.mult)
            nc.gpsimd.tensor_tensor(cv[:, idx:NM], cv[:, idx:NM], cvds[:, idx:NM],
                                    OP.add)

            # ---- maturity event ----
            if st["event"] is not None:
                ev, kslots = st["event"]
                pay = work.tile([P, G, NS], F32, tag="pay")
                sd_bc = sd_new[:].unsqueeze(-1).broadcast_to([P, G, NS])
                kd_bc = krep[:, ev * NS:(ev + 1) * NS].unsqueeze(1).broadcast_to([P, G, NS])
                nc.gpsimd.tensor_tensor(pay[:], sd_bc, kd_bc, OP.subtract)
                nc.vector.tensor_scalar(pay[:], pay[:], 0.0, None, OP.max)
                price = work.tile([P, G, NS], F32, tag="price")
                nc.gpsimd.tensor_tensor(price[:], pay[:], cv[:, idx, :, :],
                                        OP.subtract)
                price2 = work.tile([P, G, NS], F32, tag="price2")
                nc.gpsimd.tensor_tensor(price2[:], price[:], price[:], OP.mult)
                red = work.tile([P, 2 * NS], F32, tag="red")
                nc.vector.tensor_reduce(red[:, 0:NS], price[:].transpose([0, 2, 1]),
                                        mybir.AxisListType.X, OP.add)
                nc.vector.tensor_reduce(red[:, NS:2 * NS], price2[:].transpose([0, 2, 1]),
                                        mybir.AxisListType.X, OP.add)
                pred = ps_pd.tile([1, 2 * NS], F32, tag="pdps")
                nc.tensor.matmul(pred[:], ones_col[:], red[:])
                for k in kslots:
                    nc.scalar.copy(outacc[0:1, k * NS:(k + 1) * NS], pred[0:1, 0:NS])
                    nc.scalar.copy(outacc[0:1, NM * NS + k * NS:NM * NS + (k + 1) * NS],
                                   pred[0:1, NS:2 * NS])

        nc.sync.dma_start(out=out_d[:].unsqueeze(0), in_=outacc[:])

    orig = bacc.get_activation_tables
    try:
        bacc.get_activation_tables = _patched_tables_factory(
            hw_specs_mod.get_activation_tables)
        nc.compile()
    finally:
        bacc.get_activation_tables = orig
    return nc


def _prep(inputs):
    """Host-side preprocessing -> (steps, arrays-for-in_maps, shards, meta)."""
    f = lambda k: np.asarray(inputs[k], dtype=np.float32)
    S0 = float(f("S0")); rate = float(f("rate"))
    z = f("z"); zz = f("zz")
    timegrid = f("timegrid"); strikes = f("strikes")
    v0 = float(f("v0")[0]); rho = float(f("rho")[0])
    mats = np.asarray(inputs["maturities"]).astype(np.int64)

    rho_t = float(np.tanh(np.float32(rho)))
    c_t = float(np.sqrt(np.float32(1.0) - np.float32(rho_t) ** 2))
    V0 = float(1.0 / (1.0 + np.exp(-np.float32(v0))) * 0.5)
    slog0 = float(np.log(np.float32(S0)))

    days = np.round(timegrid * 365.0).astype(np.int64)
    le = days[1:, None] <= mats[None, :]
    idx_net = np.argmax(le, axis=1)
    is_mat = np.any(days[1:, None] == mats[None, :], axis=1)

    if not is_mat.any():
        return None

    T = int(np.max(np.nonzero(is_mat)[0])) + 1
    steps = []
    krep_list = []
    ev = 0
    for t in range(T):
        t0 = float(timegrid[t]); t1 = float(timegrid[t + 1])
        h = float(np.float32(t1) - np.float32(t0))
        sqh = float(np.sqrt(np.float32(h)))
        event = None
        if is_mat[t]:
            k = int(idx_net[t])
            event = (ev, [k])
            krep_list.append(np.exp(-rate * t1).astype(np.float32) * strikes)
            ev += 1
        steps.append(dict(
            t0=t0, h=h, sqh=sqh, rho_s=rho_t * sqh, c_s=c_t * sqh, rate=rate,
            idx=int(idx_net[t]), event=event,
        ))

    sv_W1 = f("sv_W1"); sv_b1 = f("sv_b1"); sv_W2 = f("sv_W2"); sv_b2 = f("sv_b2")
    sv_W3 = f("sv_W3"); sv_b3 = f("sv_b3")
    vh_W1 = f("vh_W1"); vh_b1 = f("vh_b1"); vh_W2 = f("vh_W2"); vh_b2 = f("vh_b2")
    vd_W1 = f("vd_W1"); vd_b1 = f("vd_b1"); vd_W2 = f("vd_W2"); vd_b2 = f("vd_b2")
    vv_W1 = f("vv_W1"); vv_b1 = f("vv_b1"); vv_W2 = f("vv_W2"); vv_b2 = f("vv_b2")

    t0s = timegrid[:T].astype(np.float32)

    arrs = {}
    # w1sv_tab [3, T*H1]: rows (w_slog, w_v, b1 + w_t0*t0)
    tab = np.zeros((3, T, H1), np.float32)
    tab[0] = sv_W1[1]
    tab[1] = sv_W1[2]
    tab[2] = sv_b1[None, :] + np.outer(t0s, sv_W1[0])
    arrs["w1sv_tab"] = tab.reshape(3, T * H1)

    # w1vh_tab [3, T*WB]: cols 0:80 vh (slog-in), 80:120 vdvv (v-in)
    tab = np.zeros((3, T, WB), np.float32)
    tab[0, :, 0:NM * VH] = vh_W1[:, 1, :].reshape(NM * VH)[None, :]
    tab[1, :, NM * VH:NM * VH + DV] = vd_W1[0][None, :]
    tab[1, :, NM * VH + DV:WB] = vv_W1[0][None, :]
    tab[2, :, 0:NM * VH] = (vh_b1.reshape(NM * VH)[None, :]
                            + np.outer(t0s, vh_W1[:, 0, :].reshape(NM * VH)))
    tab[2, :, NM * VH:NM * VH + DV] = vd_b1[None, :]
    tab[2, :, NM * VH + DV:WB] = vv_b1[None, :]
    arrs["w1vh_tab"] = tab.reshape(3, T * WB)

    arrs["w2aug"] = np.concatenate([sv_W2, sv_b2[None, :]], 0)
    arrs["w3b"] = np.concatenate([sv_W3, sv_b3[None, :]], 0)

    wvvh = np.zeros((WB + 1, NF + 2), np.float32)
    for k in range(NM):
        wvvh[k * VH:(k + 1) * VH, k * NS:(k + 1) * NS] = vh_W2[k]
        wvvh[WB, k * NS:(k + 1) * NS] = vh_b2[k]
    wvvh[NM * VH:NM * VH + DV, NF] = vd_W2[:, 0]
    wvvh[WB, NF] = vd_b2[0]
    wvvh[NM * VH + DV:WB, NF + 1] = vv_W2[:, 0]
    wvvh[WB, NF + 1] = vv_b2[0]
    arrs["wvvh"] = wvvh

    t1s = timegrid[1:T + 1]
    arrs["expb"] = np.ascontiguousarray((-rate * t1s)[None, :].astype(np.float32))
    if krep_list:
        arrs["krep"] = np.concatenate(krep_list)[None, :].astype(np.float32)
    else:
        arrs["krep"] = np.zeros((1, NS), np.float32)
    sd0 = float(np.exp(np.float32(slog0) - np.float32(rate) * timegrid[0]))
    arrs["initvals"] = np.array([[slog0, V0, sd0, 0.0]], np.float32)
    import ml_dtypes
    bf16_keys = {"w1sv_tab", "w1vh_tab", "w2aug", "w3b", "wvvh"}
    for k in arrs:
        dt = ml_dtypes.bfloat16 if k in bf16_keys else np.float32
        arrs[k] = np.ascontiguousarray(arrs[k].astype(dt))

    # host-prescaled noise: z_land = sqh*z ; db_land = rho_s*z + c_s*zz
    sqh_v = np.array([s["sqh"] for s in steps], np.float32)[None, :]
    rho_v = np.array([s["rho_s"] for s in steps], np.float32)[None, :]
    cs_v = np.array([s["c_s"] for s in steps], np.float32)[None, :]
    zshards, dbshards = [], []
    for c in range(N_CORES):
        zc = z[c * MCC:(c + 1) * MCC, :T]
        zzc = zz[c * MCC:(c + 1) * MCC, :T]
        zs = zc * sqh_v
        dbs = zc * rho_v + zzc * cs_v
        for src, lst in ((zs, zshards), (dbs, dbshards)):
            s = src.reshape(G, P, T).transpose(1, 2, 0)
            lst.append(np.ascontiguousarray(s, dtype=np.float32))

    written = sorted({k for s in steps if s["event"] for k in s["event"][1]})
    return steps, arrs, zshards, dbshards, written, T


_CACHE = {}


def kernel(**inputs) -> np.ndarray:
    prep = _prep(inputs)
    if prep is None:
        return np.zeros((2, NM, NS), np.float32)
    steps, arrs, zshards, dbshards, written, T = prep

    key = (T,) + tuple(
        (s["t0"], s["h"], s["rho_s"], s["c_s"], s["rate"], s["idx"],
         None if s["event"] is None else (s["event"][0], tuple(s["event"][1])))
        for s in steps)
    nc = _CACHE.get(key)
    if nc is None:
        nc = build_program(steps)
        _CACHE[key] = nc

    in_maps = []
    for c in range(N_CORES):
        m = dict(arrs)
        m["z_land"] = zshards[c]
        m["db_land"] = dbshards[c]
        in_maps.append(m)

    res = run_bass_kernel_spmd(nc, in_maps, list(range(N_CORES)))
    sums = np.zeros(2 * NM * NS, np.float64)
    for c in range(N_CORES):
        sums += res.results[c]["out"].astype(np.float64)
    s1 = sums[:NM * NS].reshape(NM, NS)
    s2 = sums[NM * NS:].reshape(NM, NS)
    pv = np.zeros((NM, NS), np.float64)
    pvar = np.zeros((NM, NS), np.float64)
    for k in written:
        pv[k] = s1[k] / MC
        pvar[k] = (s2[k] - MC * pv[k] ** 2) / (MC - 1)
    return np.stack([pv, pvar]).astype(np.float32)


# revision 17
# speedup vs baseline: 1.1666x; 1.1561x over previous
"""Trainium2 Bass kernel for nn_Net_LSV: neural local-stochastic-vol Monte Carlo.

Data-parallel over MC paths across 8 NeuronCores (2048 paths/core).
Layout per core: path p = g*128 + i -> partition i, chunk g (g in [0,16)).
Per-step pipeline:
  - state (slog, v) kept path-major [128,16] f32 + bf16 staging [128,32]
  - 8 PE transposes (bf16) -> feature-major x2 [6,1024] (slog/v/ones x 2 halves)
  - L1 (sv 100 + vh/vdvv 120 fused) via time-indexed weight tables (bias baked
    into a third input row -> relus are bias-free, runnable on any engine)
  - L2 sv 100x100, then per-chunk folds: pd ([101,128] x [101,1]) and
    vvh ([121,128] x [121,86] -> 84 hedge cols + vd + vv)
  - softplus / tamed-Euler state update spread across Act/DVE/Pool
Matmul operands bf16 (f32 psum accumulate); elementwise f32.
"""
import numpy as np
from contextlib import ExitStack

import concourse.bass as bass
import concourse.bacc as bacc
import concourse.tile as tile
from concourse import mybir
from concourse.masks import make_identity
from concourse.bass_utils import run_bass_kernel_spmd
import concourse.hw_specs as hw_specs_mod

F32 = mybir.dt.float32
BF16 = mybir.dt.bfloat16
AF = mybir.ActivationFunctionType
OP = mybir.AluOpType

N_CORES = 8
MC = 16384
P = 128
G = 16
MCC = P * G            # paths per core
NS = 21                # strikes
NM = 4                 # maturities
H1 = 100               # s_vol hidden
VH = 20                # vanilla hedge hidden (per maturity)
DV = 20                # v_drift / v_vol hidden
WB = NM * VH + 2 * DV  # fused vh+vdvv L1 width (120)
NF = NM * NS           # 84 hedge outputs
SQH_HALF = float(np.sqrt(0.5))

_ONE_TABLE = "natural_log_exp_and_others"
_MY_FNS = None


def _patched_tables_factory(orig):
    def patched(arch):
        t = orig(arch)
        if _ONE_TABLE not in t:
            return t
        global _MY_FNS
        if _MY_FNS is None:
            _MY_FNS = {
                AF.Relu, AF.Exp, AF.Ln, AF.Abs, AF.Copy, AF.Square,
                AF.Identity,
            }
        out = {}
        for k, v in t.items():
            out[k] = v if k == _ONE_TABLE else (v - _MY_FNS)
        return out
    return patched


def build_program(steps):
    T = len(steps)
    n_ev = sum(1 for s in steps if s["event"] is not None)
    nc = bacc.Bacc()

    # ---------------- DRAM I/O ----------------
    z_d = nc.declare_dram_parameter("z_land", [P, T, G], F32, isOutput=False)
    db_d = nc.declare_dram_parameter("db_land", [P, T, G], F32, isOutput=False)
    w1sv_d = nc.declare_dram_parameter("w1sv_tab", [3, T * H1], BF16, isOutput=False)
    w1vh_d = nc.declare_dram_parameter("w1vh_tab", [3, T * WB], BF16, isOutput=False)
    w2aug_d = nc.declare_dram_parameter("w2aug", [H1 + 1, H1], BF16, isOutput=False)
    w3b_d = nc.declare_dram_parameter("w3b", [H1 + 1, 1], BF16, isOutput=False)
    wvvh_d = nc.declare_dram_parameter("wvvh", [WB + 1, NF + 2], BF16, isOutput=False)
    expb_d = nc.declare_dram_parameter("expb", [1, T], F32, isOutput=False)
    krep_d = nc.declare_dram_parameter("krep", [1, max(n_ev, 1) * NS], F32, isOutput=False)
    init_d = nc.declare_dram_parameter("initvals", [1, 4], F32, isOutput=False)
    out_d = nc.declare_dram_parameter("out", [2 * NM * NS], F32, isOutput=True)

    with tile.TileContext(nc) as tc, ExitStack() as ctx:
        stat = ctx.enter_context(tc.tile_pool(name="stat", bufs=1))
        work = ctx.enter_context(tc.tile_pool(name="work", bufs=2))
        spw = ctx.enter_context(tc.tile_pool(name="spw", bufs=1))
        ps_x2 = ctx.enter_context(tc.tile_pool(name="ps_x2", bufs=1, space="PSUM"))
        ps_mlp = ctx.enter_context(tc.tile_pool(name="ps_mlp", bufs=2, space="PSUM"))
        ps_cvf = ctx.enter_context(tc.tile_pool(name="ps_cvf", bufs=2, space="PSUM"))

        # ---------- static tiles ----------
        ident_f = stat.tile([P, P], F32)
        make_identity(nc, ident_f[:])
        ident = stat.tile([P, P], BF16)
        nc.vector.tensor_copy(ident[:], ident_f[:])

        zt = stat.tile([P, T, G], F32)
        nc.sync.dma_start(out=zt[:], in_=z_d[:])
        dbt = stat.tile([P, T, G], F32)
        nc.sync.dma_start(out=dbt[:], in_=db_d[:])

        def load_bf16(dram, shape, tag):
            b = stat.tile(shape, BF16, tag=tag + "_b")
            nc.sync.dma_start(out=b[:], in_=dram[:])
            return b

        def load_bf16_dup32(dram, shape, tag):
            b = stat.tile([32 + shape[0]] + shape[1:], BF16, tag=tag + "_b")
            nc.sync.dma_start(out=b[0:shape[0]], in_=dram[:])
            nc.sync.dma_start(out=b[32:32 + shape[0]], in_=dram[:])
            return b

        w1sv = load_bf16_dup32(w1sv_d, [3, T * H1], "w1sv")
        w1vh = load_bf16_dup32(w1vh_d, [3, T * WB], "w1vh")
        w2aug = load_bf16(w2aug_d, [H1 + 1, H1], "w2aug")
        w3b = load_bf16(w3b_d, [H1 + 1, 1], "w3b")
        wvvh = load_bf16(wvvh_d, [WB + 1, NF + 2], "wvvh")

        expb = stat.tile([P, T], F32)
        nc.sync.dma_start(out=expb[:], in_=expb_d[:].broadcast_to([P, T]))
        krep = stat.tile([P, max(n_ev, 1) * NS], F32)
        nc.sync.dma_start(out=krep[:], in_=krep_d[:].broadcast_to([P, max(n_ev, 1) * NS]))
        initv = stat.tile([P, 4], F32)
        nc.sync.dma_start(out=initv[:], in_=init_d[:].broadcast_to([P, 4]))

        bias0 = stat.tile([P, 1], F32)
        nc.gpsimd.memset(bias0[:], 0.0)
        ones_col = stat.tile([P, 1], F32)
        nc.gpsimd.memset(ones_col[:], 1.0)

        # ---------- persistent state ----------
        slog = stat.tile([P, G], F32)
        nc.vector.tensor_copy(slog[:], initv[:, 0:1].broadcast_to([P, G]))
        v = stat.tile([P, G], F32)
        nc.vector.tensor_copy(v[:], initv[:, 1:2].broadcast_to([P, G]))
        sd_a = stat.tile([P, G], F32)
        nc.vector.tensor_copy(sd_a[:], initv[:, 2:3].broadcast_to([P, G]))
        sd_b = stat.tile([P, G], F32)
        nc.gpsimd.memset(sd_b[:], 0.0)
        # bf16 staging: cols 0:16 slog chunks, 16:32 v chunks
        svb = stat.tile([P, 2 * G], BF16)

        # feature-major input: rows 0:3 (slogA, vA, 1), rows 32:35 (slogB, vB, 1)
        x2sb = stat.tile([35, MCC // 2], BF16)
        nc.gpsimd.memset(x2sb[:], 1.0)

        h1sv = stat.tile([H1 + 1, MCC], BF16)
        nc.gpsimd.memset(h1sv[:], 1.0)
        h1vh = stat.tile([WB + 1, MCC], BF16)
        nc.gpsimd.memset(h1vh[:], 1.0)
        h2aug = stat.tile([H1 + 1, MCC], BF16)
        nc.gpsimd.memset(h2aug[:], 1.0)

        cvpre = stat.tile([P, NM, G, NS], BF16)
        cvfwd = stat.tile([P, NM, G, NS], BF16)
        dS = stat.tile([P, G], F32)
        cv = stat.tile([P, NM, G, NS], F32)
        nc.gpsimd.memset(cv[:], 0.0)
        vdvu = stat.tile([P, 2 * G], F32)    # cols 0:16 vd, 16:32 vv(pre-softplus)
        pd = stat.tile([P, G], F32)
        vvol = stat.tile([P, G], F32)
        outacc = stat.tile([1, 2 * NM * NS], F32)
        nc.gpsimd.memset(outacc[:], 0.0)

        sd_tiles = [sd_a, sd_b]

        # round-robin engine pickers
        eng3 = [nc.scalar, nc.vector, nc.gpsimd]
        eng2l = [nc.scalar, nc.vector]

        def relu_on(eng, out, in_, nparts):
            if eng is nc.scalar:
                nc.scalar.activation(out, in_, AF.Relu,
                                     bias=bias0[0:nparts, :], scale=1.0)
            else:
                eng.tensor_scalar(out, in_, 0.0, None, OP.max)

        def copy_on(eng, out, in_):
            if eng is nc.scalar:
                nc.scalar.copy(out, in_)
            else:
                eng.tensor_copy(out, in_)

        def sp_softplus(dst, src, nparts, tagp, abs_eng, r_eng, add_eng,
                        abs_act=False):
            """dst = softplus(src), stable: max(x,0) + ln(1+exp(-|x|))."""
            shape = list(src.shape)
            pool = spw if tagp == "sp_cv" else work
            a = pool.tile(shape, F32, tag=tagp + "_a")
            if abs_act:
                nc.scalar.activation(a[:], src, AF.Abs,
                                     bias=bias0[0:nparts, :], scale=1.0)
            else:
                abs_eng.scalar_tensor_tensor(a[:], src, -1.0, src, OP.mult, OP.max)
            e = pool.tile(shape, F32, tag=tagp + "_e")
            nc.scalar.activation(e[:], a[:], AF.Exp, bias=bias0[0:nparts, :], scale=-1.0)
            l = pool.tile(shape, F32, tag=tagp + "_l")
            nc.scalar.activation(l[:], e[:], AF.Ln, bias=ones_col[0:nparts, :], scale=1.0)
            r = pool.tile(shape, F32, tag=tagp + "_r")
            r_eng.tensor_scalar(r[:], src, 0.0, None, OP.max)
            add_eng.tensor_tensor(dst, r[:], l[:], OP.add)

        NC2 = NF + 2

        for t, st in enumerate(steps):
            h, sqh = st["h"], st["sqh"]
            rate = st["rate"]
            idx = st["idx"]
            nlive = NM - idx
            sd_old = sd_tiles[t % 2]
            sd_new = sd_tiles[(t + 1) % 2]
            z_t = zt[:, t, :]        # sqh * z  (host-scaled)
            db_t = dbt[:, t, :]      # rho_s*z + c_s*zz (host-scaled)

            # ---- bf16 staging of state ----
            nc.vector.tensor_copy(svb[:, 0:G], slog[:])
            nc.scalar.copy(svb[:, G:2 * G], v[:])

            # ---- 16 PE transposes -> x2ps [2, 2048]; 2 copies -> x2sb ----
            x2ps = ps_x2.tile([2, MCC], BF16, tag="x2ps")
            for q in range(8):
                nc.tensor.transpose(x2ps[:, q * P:(q + 1) * P],
                                    svb[:, q:2 * G:G], ident[:])
            nc.scalar.copy(x2sb[0:2, :], x2ps[:, 0:1024])
            for q in range(8):
                nc.tensor.transpose(x2ps[:, 1024 + q * P:1024 + (q + 1) * P],
                                    svb[:, 8 + q:2 * G:G], ident[:])
            nc.vector.tensor_copy(x2sb[32:34, :], x2ps[:, 1024:2048])

            # ---- L1 sv (2 half-MMs per relu), L2, pd folds, softplus(pd) ----
            for hf in range(2):
                m = ps_mlp.tile([P, 1024], F32, tag="mlp")
                base, wrow = (0, 0) if hf == 0 else (32, 32)
                nc.tensor.matmul(m[0:H1, 0:512],
                                 w1sv[wrow:wrow + 3, t * H1:(t + 1) * H1],
                                 x2sb[base:base + 3, 0:512])
                nc.tensor.matmul(m[0:H1, 512:1024],
                                 w1sv[wrow:wrow + 3, t * H1:(t + 1) * H1],
                                 x2sb[base:base + 3, 512:1024])
                relu_on(eng2l[hf % 2], h1sv[0:H1, hf * 1024:(hf + 1) * 1024],
                        m[0:H1, :], H1)
            for hf in range(2):
                m = ps_mlp.tile([P, 1024], F32, tag="mlp")
                nc.tensor.matmul(m[0:H1, 0:512], w2aug[:],
                                 h1sv[:, hf * 1024:hf * 1024 + 512])
                nc.tensor.matmul(m[0:H1, 512:1024], w2aug[:],
                                 h1sv[:, hf * 1024 + 512:(hf + 1) * 1024])
                relu_on(eng2l[(hf + 1) % 2], h2aug[0:H1, hf * 1024:(hf + 1) * 1024],
                        m[0:H1, :], H1)
            pdps = ps_cvf.tile([P, 2 * NC2], F32, tag="cvk")
            for g in range(G):
                nc.tensor.matmul(pdps[:, g:g + 1],
                                 h2aug[:, g * P:(g + 1) * P], w3b[:])
            sp_softplus(pd[:], pdps[:, 0:G], P, "sp_pd", nc.vector, nc.vector,
                        nc.vector, abs_act=True)

            # ---- Slog update chain ----
            sq = work.tile([P, G], F32, tag="sq")
            nc.scalar.activation(sq[:], pd[:], AF.Square, bias=bias0[:],
                                 scale=SQH_HALF)
            drift = work.tile([P, G], F32, tag="drift")
            nc.vector.tensor_scalar(drift[:], sq[:], -1.0, float(rate),
                                    OP.mult, OP.add)
            dc = work.tile([P, G], F32, tag="dc")
            nc.scalar.activation(dc[:], drift[:], AF.Abs, bias=bias0[:],
                                 scale=float(sqh))
            nc.vector.tensor_scalar(dc[:], dc[:], 1.0, None, OP.add)
            rcp1 = work.tile([P, G], F32, tag="rcp1")
            nc.vector.reciprocal(rcp1[:], dc[:])
            term1 = work.tile([P, G], F32, tag="term1")
            nc.vector.scalar_tensor_tensor(term1[:], drift[:], float(h), rcp1[:],
                                           OP.mult, OP.mult)
            fc = work.tile([P, G], F32, tag="fc")
            nc.vector.tensor_scalar(fc[:], pd[:], float(sqh), 1.0, OP.mult, OP.add)
            rcp2 = work.tile([P, G], F32, tag="rcp2")
            nc.vector.reciprocal(rcp2[:], fc[:])
            diffz = work.tile([P, G], F32, tag="diffz")
            nc.vector.tensor_tensor(diffz[:], z_t, pd[:], OP.mult)
            term2 = work.tile([P, G], F32, tag="term2")
            nc.vector.tensor_tensor(term2[:], diffz[:], rcp2[:], OP.mult)
            t12 = work.tile([P, G], F32, tag="t12")
            nc.vector.tensor_tensor(t12[:], term1[:], term2[:], OP.add)
            nc.vector.tensor_tensor(slog[:], slog[:], t12[:], OP.add)
            nc.scalar.activation(sd_new[:], slog[:], AF.Exp,
                                 bias=expb[:, t:t + 1], scale=1.0)
            nc.vector.tensor_tensor(dS[:], sd_new[:], sd_old[:], OP.subtract)

            # ---- vh/vdvv L1, vvh folds (2 chunks/psum tile, live cols) ----
            for hf in range(2):
                m = ps_mlp.tile([P, 1024], F32, tag="mlp")
                base, wrow = (0, 0) if hf == 0 else (32, 32)
                nc.tensor.matmul(m[0:WB, 0:512],
                                 w1vh[wrow:wrow + 3, t * WB:(t + 1) * WB],
                                 x2sb[base:base + 3, 0:512])
                nc.tensor.matmul(m[0:WB, 512:1024],
                                 w1vh[wrow:wrow + 3, t * WB:(t + 1) * WB],
                                 x2sb[base:base + 3, 512:1024])
                relu_on(eng2l[hf % 2], h1vh[0:WB, hf * 1024:(hf + 1) * 1024],
                        m[0:WB, :], WB)
            for gp in range(G // 2):
                cvk = ps_cvf.tile([P, 2 * NC2], F32, tag="cvk")
                nc.tensor.matmul(cvk[:, idx * NS:NC2],
                                 h1vh[:, (2 * gp) * P:(2 * gp + 1) * P],
                                 wvvh[:, idx * NS:NC2])
                nc.tensor.matmul(cvk[:, NC2 + idx * NS:2 * NC2],
                                 h1vh[:, (2 * gp + 1) * P:(2 * gp + 2) * P],
                                 wvvh[:, idx * NS:NC2])
                src_cv = cvk[:].rearrange("p (j f) -> p j f", j=2)[:, :, idx * NS:NF] \
                    .rearrange("p j (k s) -> p k j s", k=nlive)
                copy_on(eng2l[gp % 2], cvpre[:, idx:NM, 2 * gp:2 * gp + 2, :], src_cv)
                src_vd = cvk[:].rearrange("p (j f) -> p j f", j=2)[:, :, NF:NF + 2] \
                    .rearrange("p j d -> p d j")
                dst_vd = vdvu[:].rearrange("p (d g) -> p d g", d=2)[:, :, 2 * gp:2 * gp + 2]
                copy_on(eng2l[(gp + 1) % 2], dst_vd, src_vd)

            # ---- softplus(vv) and V update ----
            sp_softplus(vvol[:], vdvu[:, G:2 * G], P, "sp_vv", nc.vector,
                        nc.vector, nc.gpsimd)
            vtmp = work.tile([P, G], F32, tag="vtmp")
            nc.vector.scalar_tensor_tensor(vtmp[:], vdvu[:, 0:G], float(h), v[:],
                                           OP.mult, OP.add)
            vvdB = work.tile([P, G], F32, tag="vvdB")
            nc.vector.tensor_tensor(vvdB[:], vvol[:], db_t, OP.mult)
            nc.vector.tensor_tensor(v[:], vtmp[:], vvdB[:], OP.add)

            # ---- softplus(cv) in bf16, cv update ----
            sp_softplus(cvfwd[:, idx:NM].rearrange("p k g s -> p (k g s)"),
                        cvpre[:, idx:NM].rearrange("p k g s -> p (k g s)"),
                        P, "sp_cv", nc.vector, nc.vector, nc.gpsimd)
            dS_b = dS[:].unsqueeze(1).unsqueeze(-1).broadcast_to([P, nlive, G, NS])
            cvds = work.tile([P, NM, G, NS], F32, tag="cvds")
            nc.vector.tensor_tensor(cvds[:, idx:NM], cvfwd[:, idx:NM], dS_b, OP.mult)
            nc.gpsimd.tensor_tensor(cv[:, idx:NM], cv[:, idx:NM], cvds[:, idx:NM],
                                    OP.add)

            # ---- maturity event ----
            if st["event"] is not None:
                ev, kslots = st["event"]
                pay = work.tile([P, G, NS], F32, tag="pay")
                sd_bc = sd_new[:].unsqueeze(-1).broadcast_to([P, G, NS])
                kd_bc = krep[:, ev * NS:(ev + 1) * NS].unsqueeze(1).broadcast_to([P, G, NS])
                nc.gpsimd.tensor_tensor(pay[:], sd_bc, kd_bc, OP.subtract)
                nc.vector.tensor_scalar(pay[:], pay[:], 0.0, None, OP.max)
                price = work.tile([P, G, NS], F32, tag="price")
                nc.gpsimd.tensor_tensor(price[:], pay[:], cv[:, idx, :, :],
                                        OP.subtract)
                price2 = work.tile([P, G, NS], F32, tag="price2")
                nc.gpsimd.tensor_tensor(price2[:], price[:], price[:], OP.mult)
                red = work.tile([P, 2 * NS], F32, tag="red")
                nc.vector.tensor_reduce(red[:, 0:NS], price[:].transpose([0, 2, 1]),
                                        mybir.AxisListType.X, OP.add)
                nc.vector.tensor_reduce(red[:, NS:2 * NS], price2[:].transpose([0, 2, 1]),
                                        mybir.AxisListType.X, OP.add)
                pred = ps_cvf.tile([1, 2 * NC2], F32, tag="cvk")
                nc.tensor.matmul(pred[:, 0:2 * NS], ones_col[:], red[:])
                for k in kslots:
                    nc.scalar.copy(outacc[0:1, k * NS:(k + 1) * NS], pred[0:1, 0:NS])
                    nc.scalar.copy(outacc[0:1, NM * NS + k * NS:NM * NS + (k + 1) * NS],
                                   pred[0:1, NS:2 * NS])

        nc.sync.dma_start(out=out_d[:].unsqueeze(0), in_=outacc[:])

    orig = bacc.get_activation_tables
    try:
        bacc.get_activation_tables = _patched_tables_factory(
            hw_specs_mod.get_activation_tables)
        nc.compile()
    finally:
        bacc.get_activation_tables = orig
    return nc


def _prep(inputs):
    """Host-side preprocessing -> (steps, arrays-for-in_maps, shards, meta)."""
    f = lambda k: np.asarray(inputs[k], dtype=np.float32)
    S0 = float(f("S0")); rate = float(f("rate"))
    z = f("z"); zz = f("zz")
    timegrid = f("timegrid"); strikes = f("strikes")
    v0 = float(f("v0")[0]); rho = float(f("rho")[0])
    mats = np.asarray(inputs["maturities"]).astype(np.int64)

    rho_t = float(np.tanh(np.float32(rho)))
    c_t = float(np.sqrt(np.float32(1.0) - np.float32(rho_t) ** 2))
    V0 = float(1.0 / (1.0 + np.exp(-np.float32(v0))) * 0.5)
    slog0 = float(np.log(np.float32(S0)))

    days = np.round(timegrid * 365.0).astype(np.int64)
    le = days[1:, None] <= mats[None, :]
    idx_net = np.argmax(le, axis=1)
    is_mat = np.any(days[1:, None] == mats[None, :], axis=1)

    if not is_mat.any():
        return None

    T = int(np.max(np.nonzero(is_mat)[0])) + 1
    steps = []
    krep_list = []
    ev = 0
    for t in range(T):
        t0 = float(timegrid[t]); t1 = float(timegrid[t + 1])
        h = float(np.float32(t1) - np.float32(t0))
        sqh = float(np.sqrt(np.float32(h)))
        event = None
        if is_mat[t]:
            k = int(idx_net[t])
            event = (ev, [k])
            krep_list.append(np.exp(-rate * t1).astype(np.float32) * strikes)
            ev += 1
        steps.append(dict(
            t0=t0, h=h, sqh=sqh, rho_s=rho_t * sqh, c_s=c_t * sqh, rate=rate,
            idx=int(idx_net[t]), event=event,
        ))

    sv_W1 = f("sv_W1"); sv_b1 = f("sv_b1"); sv_W2 = f("sv_W2"); sv_b2 = f("sv_b2")
    sv_W3 = f("sv_W3"); sv_b3 = f("sv_b3")
    vh_W1 = f("vh_W1"); vh_b1 = f("vh_b1"); vh_W2 = f("vh_W2"); vh_b2 = f("vh_b2")
    vd_W1 = f("vd_W1"); vd_b1 = f("vd_b1"); vd_W2 = f("vd_W2"); vd_b2 = f("vd_b2")
    vv_W1 = f("vv_W1"); vv_b1 = f("vv_b1"); vv_W2 = f("vv_W2"); vv_b2 = f("vv_b2")

    t0s = timegrid[:T].astype(np.float32)

    arrs = {}
    # w1sv_tab [3, T*H1]: rows (w_slog, w_v, b1 + w_t0*t0)
    tab = np.zeros((3, T, H1), np.float32)
    tab[0] = sv_W1[1]
    tab[1] = sv_W1[2]
    tab[2] = sv_b1[None, :] + np.outer(t0s, sv_W1[0])
    arrs["w1sv_tab"] = tab.reshape(3, T * H1)

    # w1vh_tab [3, T*WB]: cols 0:80 vh (slog-in), 80:120 vdvv (v-in)
    tab = np.zeros((3, T, WB), np.float32)
    tab[0, :, 0:NM * VH] = vh_W1[:, 1, :].reshape(NM * VH)[None, :]
    tab[1, :, NM * VH:NM * VH + DV] = vd_W1[0][None, :]
    tab[1, :, NM * VH + DV:WB] = vv_W1[0][None, :]
    tab[2, :, 0:NM * VH] = (vh_b1.reshape(NM * VH)[None, :]
                            + np.outer(t0s, vh_W1[:, 0, :].reshape(NM * VH)))
    tab[2, :, NM * VH:NM * VH + DV] = vd_b1[None, :]
    tab[2, :, NM * VH + DV:WB] = vv_b1[None, :]
    arrs["w1vh_tab"] = tab.reshape(3, T * WB)

    arrs["w2aug"] = np.concatenate([sv_W2, sv_b2[None, :]], 0)
    arrs["w3b"] = np.concatenate([sv_W3, sv_b3[None, :]], 0)

    wvvh = np.zeros((WB + 1, NF + 2), np.float32)
    for k in range(NM):
        wvvh[k * VH:(k + 1) * VH, k * NS:(k + 1) * NS] = vh_W2[k]
        wvvh[WB, k * NS:(k + 1) * NS] = vh_b2[k]
    wvvh[NM * VH:NM * VH + DV, NF] = vd_W2[:, 0]
    wvvh[WB, NF] = vd_b2[0]
    wvvh[NM * VH + DV:WB, NF + 1] = vv_W2[:, 0]
    wvvh[WB, NF + 1] = vv_b2[0]
    arrs["wvvh"] = wvvh

    t1s = timegrid[1:T + 1]
    arrs["expb"] = np.ascontiguousarray((-rate * t1s)[None, :].astype(np.float32))
    if krep_list:
        arrs["krep"] = np.concatenate(krep_list)[None, :].astype(np.float32)
    else:
        arrs["krep"] = np.zeros((1, NS), np.float32)
    sd0 = float(np.exp(np.float32(slog0) - np.float32(rate) * timegrid[0]))
    arrs["initvals"] = np.array([[slog0, V0, sd0, 0.0]], np.float32)
    import ml_dtypes
    bf16_keys = {"w1sv_tab", "w1vh_tab", "w2aug", "w3b", "wvvh"}
    for k in arrs:
        dt = ml_dtypes.bfloat16 if k in bf16_keys else np.float32
        arrs[k] = np.ascontiguousarray(arrs[k].astype(dt))

    # host-prescaled noise: z_land = sqh*z ; db_land = rho_s*z + c_s*zz
    sqh_v = np.array([s["sqh"] for s in steps], np.float32)[None, :]
    rho_v = np.array([s["rho_s"] for s in steps], np.float32)[None, :]
    cs_v = np.array([s["c_s"] for s in steps], np.float32)[None, :]
    zshards, dbshards = [], []
    for c in range(N_CORES):
        zc = z[c * MCC:(c + 1) * MCC, :T]
        zzc = zz[c * MCC:(c + 1) * MCC, :T]
        zs = zc * sqh_v
        dbs = zc * rho_v + zzc * cs_v
        for src, lst in ((zs, zshards), (dbs, dbshards)):
            s = src.reshape(G, P, T).transpose(1, 2, 0)
            lst.append(np.ascontiguousarray(s, dtype=np.float32))

    written = sorted({k for s in steps if s["event"] for k in s["event"][1]})
    return steps, arrs, zshards, dbshards, written, T


_CACHE = {}


def kernel(**inputs) -> np.ndarray:
    prep = _prep(inputs)
    if prep is None:
        return np.zeros((2, NM, NS), np.float32)
    steps, arrs, zshards, dbshards, written, T = prep

    key = (T,) + tuple(
        (s["t0"], s["h"], s["rho_s"], s["c_s"], s["rate"], s["idx"],
         None if s["event"] is None else (s["event"][0], tuple(s["event"][1])))
        for s in steps)
    nc = _CACHE.get(key)
    if nc is None:
        nc = build_program(steps)
        _CACHE[key] = nc

    in_maps = []
    for c in range(N_CORES):
        m = dict(arrs)
        m["z_land"] = zshards[c]
        m["db_land"] = dbshards[c]
        in_maps.append(m)

    res = run_bass_kernel_spmd(nc, in_maps, list(range(N_CORES)))
    sums = np.zeros(2 * NM * NS, np.float64)
    for c in range(N_CORES):
        sums += res.results[c]["out"].astype(np.float64)
    s1 = sums[:NM * NS].reshape(NM, NS)
    s2 = sums[NM * NS:].reshape(NM, NS)
    pv = np.zeros((NM, NS), np.float64)
    pvar = np.zeros((NM, NS), np.float64)
    for k in written:
        pv[k] = s1[k] / MC
        pvar[k] = (s2[k] - MC * pv[k] ** 2) / (MC - 1)
    return np.stack([pv, pvar]).astype(np.float32)
